# revision 21
# baseline (speedup 1.0000x reference)
"""Chamfer distance kernel for Trainium2 (8 NeuronCores, SPMD).

Problem: xyz1 [4, 8192, 3], xyz2 [4, 8192, 3] (fp32 randn)
  d1[b, n] = min_m ||xyz1[b,n] - xyz2[b,m]||^2
  d2[b, m] = min_n ||xyz1[b,n] - xyz2[b,m]||^2
Returns (d1, d2), both [4, 8192] fp32.

Sharding: 8 cores = (batch b in 0..3) x (half h in 0..1).  Core (b, h)
handles queries n in [h*4096, (h+1)*4096) of batch b against the full
xyz2[b]:
  - d1 for its 4096 queries (exact),
  - a d2 partial = min over its n-half for all 8192 m (host combines
    the two halves with np.minimum).

Device algorithm (per core):
  Augmented K=24 bf16 matmul computes  psum = q.d - 0.5||d||^2
  - 0.5||q||^2 = -dist/2 on the tensor engine.  fp32 coords are split
  into three bf16 terms (q = q0+q1+q2, exact: 8+8+8 mantissa bits) and
  the six significant cross products (q0d0, q0d1, q1d0, q0d2, q2d0,
  q1d1) plus bf16-split norm rows reconstruct the fp32 dot product to
  ~2^-24 while streaming at full bf16 rate (1 cycle/row; fp32 matmul
  would be 4x slower).  ScalarE evacuates PSUM->SBUF as fp16 dist
  (Copy with scale=-2).  VectorE folds fp16 tiles with
  tensor_tensor(min) in 2x mode along both directions:
    - d1: fold the 4 m-groups per n-tile, then row-reduce,
    - d2: fold the 32 n-tiles into persistent per-column accumulators,
      then PE-transpose + row-reduce for the cross-partition min.
"""

import ml_dtypes
import numpy as np

import concourse.bass as bass
import concourse.mybir as mybir
import concourse.tile as tile
from concourse import bacc
from concourse.bass_utils import run_bass_kernel_spmd

B, N, M = 4, 8192, 8192
NCORES = 8
QH = N // 2          # queries per core (4096)
NT = QH // 128       # 32 n-tiles of 128 queries
GW = 2048            # psum group width (4 banks)
NG = M // GW         # 4 groups per n-tile
NCH = GW // 512      # 4 matmul chunks (1 psum bank each) per group

K = 24               # augmented contraction rows (bf16 triple-split)

F16 = mybir.dt.float16
F32 = mybir.dt.float32
BF16 = mybir.dt.bfloat16
MIN = mybir.AluOpType.min
AXX = mybir.AxisListType.X
NPBF = ml_dtypes.bfloat16

_cached = {}


def build_bass():
    nc = bacc.Bacc("TRN2", target_bir_lowering=False, debug=False)
    w_d = nc.dram_tensor("w", [K, QH], BF16, kind="ExternalInput").ap()
    rhs_d = nc.dram_tensor("rhs", [K, M], BF16, kind="ExternalInput").ap()
    id_d = nc.dram_tensor("ident", [128, 128], F16, kind="ExternalInput").ap()
    d1_d = nc.dram_tensor("d1", [128, NT], F32, kind="ExternalOutput").ap()
    d2_d = nc.dram_tensor("d2", [128, M // 128], F32, kind="ExternalOutput").ap()

    with tile.TileContext(nc) as tc:
        with tc.tile_pool(name="persist", bufs=1) as pp:
            w_s = pp.tile([K, QH], BF16, tag="w_s")
            rhs_s = pp.tile([K, M], BF16, tag="rhs_s")
            id_s = pp.tile([128, 128], F16, tag="id_s")
            d1b = pp.tile([128, NT], F32, tag="d1b")
            d2b = pp.tile([128, M // 128], F32, tag="d2b")
            acc2 = [
                pp.tile([128, M], F16, tag="acc2_0", name="acc2_0"),
                pp.tile([128, M], F16, tag="acc2_1", name="acc2_1"),
            ]
            nc.sync.dma_start(w_s[:], w_d)
            nc.sync.dma_start(rhs_s[:], rhs_d)
            nc.sync.dma_start(id_s[:], id_d)

            # Dummy 1-wait matmuls: absorb each input-DMA semaphore into
            # PE's observed clock so real matmuls never wait on DMA
            # (matmul ISA struct encodes at most one sync wait).
            with tc.tile_pool(name="dummy", bufs=1, space="PSUM") as dup:
                dm1 = dup.tile([1, 8], F32, tag="dm1", name="dm1")
                dm2 = dup.tile([1, 8], F32, tag="dm2", name="dm2")
                dm3 = dup.tile([1, 8], F32, tag="dm3", name="dm3")
                nc.tensor.matmul(dm1[0:1, 0:1], w_s[0:1, 0:1], w_s[0:1, 0:1])
                nc.tensor.matmul(dm2[0:1, 0:1], rhs_s[0:1, 0:1], rhs_s[0:1, 0:1])
                nc.tensor.matmul(dm3[0:1, 0:1], id_s[0:1, 0:1], id_s[0:1, 0:1])

            NCHUNK = M // 512  # 16 chunks of 512 per n-tile
            with (
                tc.tile_pool(name="psum", bufs=8, space="PSUM") as psp,
                tc.tile_pool(name="sp", bufs=4) as sp,
                tc.tile_pool(name="a1p", bufs=4) as a1p,
            ):
                for t in range(NT):
                    lhsT = w_s[:, t * 128 : (t + 1) * 128]
                    a1 = None
                    for ci in range(NCHUNK):
                        pt = psp.tile([128, 512], F32, tag="pt", name="pt")
                        nc.tensor.matmul(
                            pt[:],
                            lhsT,
                            rhs_s[:, ci * 512 : (ci + 1) * 512],
                            start=True,
                            stop=True,
                        )
                        # fp16 dist tile = -2 * psum
                        if ci == 0:
                            s = a1p.tile([128, 512], F16, tag="a1", name="s0")
                        else:
                            s = sp.tile([128, 512], F16, tag="s", name="s")
                        nc.scalar.mul(s[:], pt[:], -2.0)
                        # d1 fold across m-chunks
                        if ci == 0:
                            a1 = s
                        else:
                            a1n = a1p.tile([128, 512], F16, tag="a1", name="a1n")
                            nc.vector.tensor_tensor(a1n[:], a1[:], s[:], MIN)
                            a1 = a1n
                        # d2 fold across n-tiles (ping-pong)
                        gs = slice(ci * 512, (ci + 1) * 512)
                        if t == 0:
                            nc.vector.tensor_copy(acc2[0][:, gs], s[:])
                        else:
                            nc.vector.tensor_tensor(
                                acc2[t % 2][:, gs], acc2[(t + 1) % 2][:, gs], s[:], MIN
                            )
                    nc.vector.tensor_reduce(
                        d1b[:, t : t + 1], a1[:], axis=AXX, op=MIN
                    )

                # d2 finish: transpose acc2 in 128-col blocks (PE) and
                # row-reduce.  Same psum pool/tag as the matmuls — a pool
                # boundary would attach multi-wait release deps to the
                # transposes (over the matmul ISA sync-wait budget).
                accf = acc2[(NT - 1) % 2]
                for blk in range(M // 128):
                    tp = psp.tile([128, 128], F16, tag="pt", name="tp")
                    nc.tensor.transpose(
                        tp[:], accf[:, blk * 128 : (blk + 1) * 128], id_s[:]
                    )
                    nc.vector.tensor_reduce(
                        d2b[:, blk : blk + 1], tp[:], axis=AXX, op=MIN
                    )

            nc.sync.dma_start(d1_d, d1b[:])
            nc.sync.dma_start(d2_d, d2b[:])
    nc.compile()
    return nc


def _split3(x):
    """Exact 3-way bf16 split of fp32 data: x ~= s0 + s1 + s2."""
    x = np.asarray(x, np.float32)
    s0 = x.astype(NPBF)
    r1 = x - s0.astype(np.float32)
    s1 = r1.astype(NPBF)
    r2 = r1 - s1.astype(np.float32)
    s2 = r2.astype(NPBF)
    return s0, s1, s2


def _aug(pts, n_norm_sign, coord_rows, norm_rows):
    """Build the [24, npts] bf16 augmented matrix.

    coord_rows: list of 6 split-indices for the 6 coord-row triples.
    norm_rows: 'ones_then_norm' (rows 18-20 ones, 21-23 norm splits) or
               'norm_then_ones'.
    The norm value used is n_norm_sign * 0.5 * ||p||^2.
    """
    npts = pts.shape[0]
    s = _split3(pts.T)  # each [3, npts]
    out = np.zeros((K, npts), dtype=NPBF)
    for i, si in enumerate(coord_rows):
        out[3 * i : 3 * i + 3] = s[si]
    norm = (pts.astype(np.float64) ** 2).sum(-1) * 0.5
    n0, n1, n2 = _split3((n_norm_sign * norm).astype(np.float32))
    if norm_rows == "ones_then_norm":
        out[18:21] = np.asarray(1.0, NPBF)
        out[21] = n0
        out[22] = n1
        out[23] = n2
    else:
        out[18] = n0
        out[19] = n1
        out[20] = n2
        out[21:24] = np.asarray(-1.0, NPBF)
    return out


def make_inputs(xyz1, xyz2):
    """Per-core augmented input arrays.

    psum = sum_k W[k,n] * RHS[k,m]
         = (q0+q1+q2).(d0+d1+d2) [6 leading terms]
           - 0.5||d||^2 - 0.5||q||^2  =  -dist/2
    Pairings (row triples): W q0,q0,q1,q0,q2,q1 x RHS d0,d1,d0,d2,d0,d1.
    Rows 18-20: W ones x RHS -0.5||d||^2 splits.
    Rows 21-23: W +0.5||q||^2 splits x RHS -ones... (sign folded: W
    carries +0.5||q||^2 and RHS carries -1).
    """
    ident = np.eye(128, dtype=np.float16)
    in_maps = []
    for c in range(NCORES):
        b, h = divmod(c, 2)
        q = xyz1[b, h * QH : (h + 1) * QH]  # [4096, 3]
        d = xyz2[b]  # [8192, 3]
        w = _aug(q, +1.0, [0, 0, 1, 0, 2, 1], "ones_then_norm")
        # W norm rows 21-23 hold +0.5||q||^2 splits; ones rows are 18-20.
        r = _aug(d, -1.0, [0, 1, 0, 2, 0, 1], "norm_then_ones")
        in_maps.append({"w": w, "rhs": r, "ident": ident})
    return in_maps


def get_runner():
    """Build the Bass program once and wrap it in a cached jitted
    shard_map executable over the 8 cores.

    Returns (run, out_info) where run(concat_inputs: list[np.ndarray])
    -> list of per-core output dicts.
    """
    if "runner" in _cached:
        return _cached["runner"]

    import jax
    from jax.sharding import Mesh, PartitionSpec
    from jax.experimental.shard_map import shard_map
    from concourse import bass2jax, mybir as mb

    bass2jax.install_neuronx_cc_hook()
    nc = build_bass()

    part_name = nc.partition_id_tensor.name if nc.partition_id_tensor else None
    in_names, out_names, out_avals, zero_outs = [], [], [], []
    for alloc in nc.m.functions[0].allocations:
        if not isinstance(alloc, mb.MemoryLocationSet):
            continue
        name = alloc.memorylocations[0].name
        if alloc.kind == "ExternalInput":
            if name != part_name:
                in_names.append(name)
        elif alloc.kind == "ExternalOutput":
            out_names.append(name)
            shape = tuple(alloc.tensor_shape)
            dtype = mb.dt.np(alloc.dtype)
            out_avals.append(jax.core.ShapedArray(shape, dtype))
            zero_outs.append(np.zeros(shape, dtype))
    n_params = len(in_names)
    n_outs = len(out_names)
    all_in_names = in_names + out_names
    if part_name is not None:
        all_in_names = all_in_names + [part_name]

    def _body(*args):
        operands = list(args)
        if part_name is not None:
            operands.append(bass2jax.partition_id_tensor())
        outs = bass2jax._bass_exec_p.bind(
            *operands,
            out_avals=tuple(out_avals),
            in_names=tuple(all_in_names),
            out_names=tuple(out_names),
            lowering_input_output_aliases=(),
            sim_require_finite=True,
            sim_require_nnan=True,
            nc=nc,
        )
        return tuple(outs)

    devices = jax.devices()[:NCORES]
    mesh = Mesh(np.asarray(devices), ("core",))
    donate = tuple(range(n_params, n_params + n_outs))
    sharded = jax.jit(
        shard_map(
            _body,
            mesh=mesh,
            in_specs=(PartitionSpec("core"),) * (n_params + n_outs),
            out_specs=(PartitionSpec("core"),) * n_outs,
            check_rep=False,
        ),
        donate_argnums=donate,
        keep_unused=True,
    )

    def run(in_maps):
        per_core = [[np.asarray(m[nm]) for nm in in_names] for m in in_maps]
        concat_in = [
            np.concatenate([per_core[c][i] for c in range(NCORES)], axis=0)
            for i in range(n_params)
        ]
        concat_zeros = [
            np.zeros((NCORES * z.shape[0], *z.shape[1:]), z.dtype)
            for z in zero_outs
        ]
        out_arrs = sharded(*concat_in, *concat_zeros)
        return [
            {
                name: np.asarray(out_arrs[i]).reshape(
                    NCORES, *out_avals[i].shape
                )[c]
                for i, name in enumerate(out_names)
            }
            for c in range(NCORES)
        ]

    _cached["runner"] = (run, (in_names, out_names, out_avals, zero_outs, sharded))
    return _cached["runner"]


def assemble(results):
    d1 = np.empty((B, N), dtype=np.float32)
    d2 = np.empty((B, M), dtype=np.float32)
    d2p = []
    for c in range(NCORES):
        b, h = divmod(c, 2)
        out = results[c]
        d1[b, h * QH : (h + 1) * QH] = out["d1"].T.reshape(QH)
        d2p.append(out["d2"].T.reshape(M))
    for b in range(B):
        d2[b] = np.minimum(d2p[2 * b], d2p[2 * b + 1])
    return d1, d2


def kernel(xyz1, xyz2):
    xyz1 = np.asarray(xyz1, dtype=np.float32)
    xyz2 = np.asarray(xyz2, dtype=np.float32)
    run, _ = get_runner()
    results = run(make_inputs(xyz1, xyz2))
    return assemble(results)


# revision 30
# speedup vs baseline: 1.9908x; 1.9908x over previous
"""Chamfer distance kernel for Trainium2 (8 NeuronCores, SPMD).

Problem: xyz1 [4, 8192, 3], xyz2 [4, 8192, 3] (fp32 randn)
  d1[b, n] = min_m ||xyz1[b,n] - xyz2[b,m]||^2
  d2[b, m] = min_n ||xyz1[b,n] - xyz2[b,m]||^2
Returns (d1, d2), both [4, 8192] fp32.

Sharding: 8 cores = (batch b in 0..3) x (half h in 0..1).  Core (b, h)
handles queries n in [h*4096, (h+1)*4096) of batch b against the full
xyz2[b]:
  - d1 for its 4096 queries (exact),
  - a d2 partial = min over its n-half for all 8192 m (host combines
    the two halves with np.minimum).

Device algorithm (per core):
  Augmented K=24 bf16 matmul computes  psum = q.d - 0.5||d||^2
  - 0.5||q||^2 = -dist/2 on the tensor engine.  fp32 coords are split
  into three bf16 terms (q = q0+q1+q2, exact: 8+8+8 mantissa bits) and
  the six significant cross products (q0d0, q0d1, q1d0, q0d2, q2d0,
  q1d1) plus bf16-split norm rows reconstruct the fp32 dot product to
  ~2^-24 while streaming at full bf16 rate (1 cycle/row; fp32 matmul
  would be 4x slower).  ScalarE evacuates PSUM->SBUF as fp16 dist
  (Copy with scale=-2).  VectorE folds fp16 tiles with
  tensor_tensor(min) in 2x mode along both directions:
    - d1: fold the 4 m-groups per n-tile, then row-reduce,
    - d2: fold the 32 n-tiles into persistent per-column accumulators,
      then PE-transpose + row-reduce for the cross-partition min.
"""

import ml_dtypes
import numpy as np

import concourse.bass as bass
import concourse.mybir as mybir
import concourse.tile as tile
from concourse import bacc
from concourse.bass_utils import run_bass_kernel_spmd

B, N, M = 4, 8192, 8192
NCORES = 8
QH = N // 2          # queries per core (4096)
NT = QH // 128       # 32 n-tiles of 128 queries
GW = 2048            # psum group width (4 banks)
NG = M // GW         # 4 groups per n-tile
NCH = GW // 512      # 4 matmul chunks (1 psum bank each) per group

K = 24               # augmented contraction rows (bf16 triple-split)

F16 = mybir.dt.float16
F32 = mybir.dt.float32
BF16 = mybir.dt.bfloat16
MIN = mybir.AluOpType.min
AXX = mybir.AxisListType.X
NPBF = ml_dtypes.bfloat16

_cached = {}


def build_bass(nt=NT):
    nc = bacc.Bacc("TRN2", target_bir_lowering=False, debug=False)
    w_d = nc.dram_tensor("w", [K, QH], BF16, kind="ExternalInput").ap()
    rhs_d = nc.dram_tensor("rhs", [K, M], BF16, kind="ExternalInput").ap()
    id_d = nc.dram_tensor("ident", [128, 128], F16, kind="ExternalInput").ap()
    d1_d = nc.dram_tensor("d1", [128, NT], F32, kind="ExternalOutput").ap()
    d2_d = nc.dram_tensor("d2", [128, M // 128], F32, kind="ExternalOutput").ap()

    with tile.TileContext(nc) as tc:
        with tc.tile_pool(name="persist", bufs=1) as pp:
            w_s = pp.tile([K, QH], BF16, tag="w_s")
            rhs_s = pp.tile([K, M], BF16, tag="rhs_s")
            id_s = pp.tile([128, 128], F16, tag="id_s")
            d1b = pp.tile([128, NT], F32, tag="d1b")
            d2b = pp.tile([128, M // 128], F32, tag="d2b")
            acc2 = [
                pp.tile([128, M], F16, tag="acc2_0", name="acc2_0"),
                pp.tile([128, M], F16, tag="acc2_1", name="acc2_1"),
            ]
            nc.sync.dma_start(w_s[:], w_d)
            nc.sync.dma_start(rhs_s[:], rhs_d)
            nc.sync.dma_start(id_s[:], id_d)

            # Dummy 1-wait matmuls: absorb each input-DMA semaphore into
            # PE's observed clock so real matmuls never wait on DMA
            # (matmul ISA struct encodes at most one sync wait).
            with tc.tile_pool(name="dummy", bufs=1, space="PSUM") as dup:
                dm1 = dup.tile([1, 8], F32, tag="dm1", name="dm1")
                dm2 = dup.tile([1, 8], F32, tag="dm2", name="dm2")
                dm3 = dup.tile([1, 8], F32, tag="dm3", name="dm3")
                nc.tensor.matmul(dm1[0:1, 0:1], w_s[0:1, 0:1], w_s[0:1, 0:1])
                nc.tensor.matmul(dm2[0:1, 0:1], rhs_s[0:1, 0:1], rhs_s[0:1, 0:1])
                nc.tensor.matmul(dm3[0:1, 0:1], id_s[0:1, 0:1], id_s[0:1, 0:1])

            NCHUNK = M // 512  # 16 chunks of 512 per n-tile
            with (
                tc.tile_pool(name="psum", bufs=8, space="PSUM") as psp,
                tc.tile_pool(name="sp", bufs=4) as sp,
                tc.tile_pool(name="a1p", bufs=4) as a1p,
            ):
                for t in range(nt):
                    lhsT = w_s[:, t * 128 : (t + 1) * 128]
                    a1 = None
                    for ci in range(NCHUNK):
                        pt = psp.tile([128, 512], F32, tag="pt", name="pt")
                        nc.tensor.matmul(
                            pt[:],
                            lhsT,
                            rhs_s[:, ci * 512 : (ci + 1) * 512],
                            start=True,
                            stop=True,
                        )
                        # fp16 dist tile = -2 * psum
                        if ci == 0:
                            s = a1p.tile([128, 512], F16, tag="a1", name="s0")
                        else:
                            s = sp.tile([128, 512], F16, tag="s", name="s")
                        nc.scalar.mul(s[:], pt[:], -2.0)
                        # d1 fold across m-chunks
                        if ci == 0:
                            a1 = s
                        else:
                            a1n = a1p.tile([128, 512], F16, tag="a1", name="a1n")
                            nc.vector.tensor_tensor(a1n[:], a1[:], s[:], MIN)
                            a1 = a1n
                        # d2 fold across n-tiles (ping-pong)
                        gs = slice(ci * 512, (ci + 1) * 512)
                        if t == 0:
                            nc.vector.tensor_copy(acc2[0][:, gs], s[:])
                        else:
                            nc.vector.tensor_tensor(
                                acc2[t % 2][:, gs], acc2[(t + 1) % 2][:, gs], s[:], MIN
                            )
                    nc.vector.tensor_reduce(
                        d1b[:, t : t + 1], a1[:], axis=AXX, op=MIN
                    )

                # d2 finish: transpose acc2 in 128-col blocks (PE) and
                # row-reduce.  Same psum pool/tag as the matmuls — a pool
                # boundary would attach multi-wait release deps to the
                # transposes (over the matmul ISA sync-wait budget).
                accf = acc2[(nt - 1) % 2]
                for blk in range(M // 128):
                    tp = psp.tile([128, 128], F16, tag="pt", name="tp")
                    nc.tensor.transpose(
                        tp[:], accf[:, blk * 128 : (blk + 1) * 128], id_s[:]
                    )
                    nc.vector.tensor_reduce(
                        d2b[:, blk : blk + 1], tp[:], axis=AXX, op=MIN
                    )

            nc.sync.dma_start(d1_d, d1b[:])
            nc.sync.dma_start(d2_d, d2b[:])
    nc.compile()
    return nc


def _split3(x):
    """Exact 3-way bf16 split of fp32 data: x ~= s0 + s1 + s2."""
    x = np.asarray(x, np.float32)
    s0 = x.astype(NPBF)
    r1 = x - s0.astype(np.float32)
    s1 = r1.astype(NPBF)
    r2 = r1 - s1.astype(np.float32)
    s2 = r2.astype(NPBF)
    return s0, s1, s2


def _aug(pts, n_norm_sign, coord_rows, norm_rows):
    """Build the [24, npts] bf16 augmented matrix.

    coord_rows: list of 6 split-indices for the 6 coord-row triples.
    norm_rows: 'ones_then_norm' (rows 18-20 ones, 21-23 norm splits) or
               'norm_then_ones'.
    The norm value used is n_norm_sign * 0.5 * ||p||^2.
    """
    npts = pts.shape[0]
    s = _split3(pts.T)  # each [3, npts]
    out = np.zeros((K, npts), dtype=NPBF)
    for i, si in enumerate(coord_rows):
        out[3 * i : 3 * i + 3] = s[si]
    norm = (pts.astype(np.float64) ** 2).sum(-1) * 0.5
    n0, n1, n2 = _split3((n_norm_sign * norm).astype(np.float32))
    if norm_rows == "ones_then_norm":
        out[18:21] = np.asarray(1.0, NPBF)
        out[21] = n0
        out[22] = n1
        out[23] = n2
    else:
        out[18] = n0
        out[19] = n1
        out[20] = n2
        out[21:24] = np.asarray(-1.0, NPBF)
    return out


def make_inputs(xyz1, xyz2):
    """Per-core augmented input arrays.

    psum = sum_k W[k,n] * RHS[k,m]
         = (q0+q1+q2).(d0+d1+d2) [6 leading terms]
           - 0.5||d||^2 - 0.5||q||^2  =  -dist/2
    Pairings (row triples): W q0,q0,q1,q0,q2,q1 x RHS d0,d1,d0,d2,d0,d1.
    Rows 18-20: W ones x RHS -0.5||d||^2 splits.
    Rows 21-23: W +0.5||q||^2 splits x RHS -ones... (sign folded: W
    carries +0.5||q||^2 and RHS carries -1).
    """
    ident = np.eye(128, dtype=np.float16)
    in_maps = []
    for c in range(NCORES):
        b, h = divmod(c, 2)
        q = xyz1[b, h * QH : (h + 1) * QH]  # [4096, 3]
        d = xyz2[b]  # [8192, 3]
        w = _aug(q, +1.0, [0, 0, 1, 0, 2, 1], "ones_then_norm")
        # W norm rows 21-23 hold +0.5||q||^2 splits; ones rows are 18-20.
        r = _aug(d, -1.0, [0, 1, 0, 2, 0, 1], "norm_then_ones")
        in_maps.append({"w": w, "rhs": r, "ident": ident})
    return in_maps


def get_runner(nt=NT):
    """Build the Bass program once and wrap it in a cached jitted
    shard_map executable over the 8 cores.

    Returns (run, out_info) where run(concat_inputs: list[np.ndarray])
    -> list of per-core output dicts.
    """
    ckey = ("runner", nt)
    if ckey in _cached:
        return _cached[ckey]

    import jax
    from jax.sharding import Mesh, PartitionSpec
    from jax.experimental.shard_map import shard_map
    from concourse import bass2jax, mybir as mb

    bass2jax.install_neuronx_cc_hook()
    nc = build_bass()

    part_name = nc.partition_id_tensor.name if nc.partition_id_tensor else None
    in_names, out_names, out_avals, zero_outs = [], [], [], []
    for alloc in nc.m.functions[0].allocations:
        if not isinstance(alloc, mb.MemoryLocationSet):
            continue
        name = alloc.memorylocations[0].name
        if alloc.kind == "ExternalInput":
            if name != part_name:
                in_names.append(name)
        elif alloc.kind == "ExternalOutput":
            out_names.append(name)
            shape = tuple(alloc.tensor_shape)
            dtype = mb.dt.np(alloc.dtype)
            out_avals.append(jax.core.ShapedArray(shape, dtype))
            zero_outs.append(np.zeros(shape, dtype))
    n_params = len(in_names)
    n_outs = len(out_names)
    all_in_names = in_names + out_names
    if part_name is not None:
        all_in_names = all_in_names + [part_name]

    def _body(*args):
        operands = list(args)
        if part_name is not None:
            operands.append(bass2jax.partition_id_tensor())
        outs = bass2jax._bass_exec_p.bind(
            *operands,
            out_avals=tuple(out_avals),
            in_names=tuple(all_in_names),
            out_names=tuple(out_names),
            lowering_input_output_aliases=(),
            sim_require_finite=True,
            sim_require_nnan=True,
            nc=nc,
        )
        return tuple(outs)

    devices = jax.devices()[:NCORES]
    mesh = Mesh(np.asarray(devices), ("core",))
    donate = tuple(range(n_params, n_params + n_outs))
    smapped = shard_map(
        _body,
        mesh=mesh,
        in_specs=(PartitionSpec("core"),) * (n_params + n_outs),
        out_specs=(PartitionSpec("core"),) * n_outs,
        check_rep=False,
    )
    sharded = jax.jit(smapped, donate_argnums=donate, keep_unused=True)

    def run(in_maps):
        per_core = [[np.asarray(m[nm]) for nm in in_names] for m in in_maps]
        concat_in = [
            np.concatenate([per_core[c][i] for c in range(NCORES)], axis=0)
            for i in range(n_params)
        ]
        concat_zeros = [
            np.zeros((NCORES * z.shape[0], *z.shape[1:]), z.dtype)
            for z in zero_outs
        ]
        out_arrs = sharded(*concat_in, *concat_zeros)
        return [
            {
                name: np.asarray(out_arrs[i]).reshape(
                    NCORES, *out_avals[i].shape
                )[c]
                for i, name in enumerate(out_names)
            }
            for c in range(NCORES)
        ]

    _cached[ckey] = (
        run,
        (in_names, out_names, out_avals, zero_outs, sharded, smapped),
    )
    return _cached[ckey]


def assemble(results):
    d1 = np.empty((B, N), dtype=np.float32)
    d2 = np.empty((B, M), dtype=np.float32)
    d2p = []
    for c in range(NCORES):
        b, h = divmod(c, 2)
        out = results[c]
        d1[b, h * QH : (h + 1) * QH] = out["d1"].T.reshape(QH)
        d2p.append(out["d2"].T.reshape(M))
    for b in range(B):
        d2[b] = np.minimum(d2p[2 * b], d2p[2 * b + 1])
    return d1, d2


def kernel(xyz1, xyz2):
    xyz1 = np.asarray(xyz1, dtype=np.float32)
    xyz2 = np.asarray(xyz2, dtype=np.float32)
    run, _ = get_runner()
    results = run(make_inputs(xyz1, xyz2))
    return assemble(results)
